# revision 1
# baseline (speedup 1.0000x reference)
"""Causal self-attention with RoPE on 8 TRN2 NeuronCores.

Sharding: pure data parallel over batch B=8 (one batch element per core,
weights replicated, no collectives).

Per-core dataflow, all matmuls bf16 with fp32 PSUM accumulation. The host
pre-transposes x, pre-packs W_attn by head-pair, pre-broadcasts biases and
pre-signs the sin table. The emission is software-pipelined per head-pair
hp: ACT exponentials for pair hp-1 stream underneath the qkv matmuls of
pair hp.

  xT (host transpose)                DMA                       [C, T]
  q^T,k^T = W_qk^T @ x + b           PE (W stationary), copies [ch, T]
                                     with fused bias on DVE
  v natural = x @ W_v + b            PE, DVE bias-add          [T, ch]
  RoPE(q,k) in place                 DVE only: stream_shuffle pair-swap,
                                     q*cos + swap(q)*signed_sin
  s^T = k @ q^T (per head)           PE K=64                   [Tk, Tq]
  p = exp(s/8), causal mask added    ACT exp <=1024 wide; mask via one
    on the diagonal block            PE matmul (idn @ mneg) into PSUM
  [y'; r]^T = [v, 1]^T @ p           PE K=128 accum            [65, Tq]
  y^T = y'^T * (1/r)                 r: ACT copy -> PE ones-broadcast ->
                                     DVE reciprocal -> DVE mul (odd heads
                                     reach partitions 64:127 via SB->SB DMA)
  out = y @ W_proj + b               PE, DVE bias-add          [T, C] f32

PSUM (8 banks exactly): ps5 shared 4x[128,512] rotation (qkv/v/narrow
scores/y'/proj accumulators) + stw 2x[128,1024] (wide scores, shared with
the reciprocal-broadcast tiles).

HW notes: GPSIMD tensor ops corrupt the first post-load execution (kernel()
therefore runs the NEFF twice and returns the steady-state result); DVE
memsets on this toolchain miscompile (constants arrive via host DMAs);
fp32r was abandoned entirely so no producer-rounding verifier rules apply.
"""
import sys

sys.path.insert(0, "/opt/trn_rl_repo")

import numpy as np

B, T, C = 8, 1024, 768
H, D = 12, 64
N_CORES = 8
KC = C // 128  # 6 K-chunks of the C contraction
NT = T // 128  # 8 T-chunks

# wa group offsets (in columns of the host-packed [128, KC, ...] layout):
# group 0: vA (384 cols), group 1: vB (384), groups 2..7: pair hp (256)
_WAOFF = [0, 384, 768, 1024, 1280, 1536, 1792, 2048]  # start col of each group
_WATOT = 2304  # total packed columns

_prog = None  # cached compiled Bass program
_DEBUG = False  # add intermediate-dump DMAs (qkT, v_sb, yT)


def _emit_body(nc, tc, dr, phases=(1, 2, 3)):
    """Emit one full forward pass. dr = dict of DRAM tensors."""
    from concourse import mybir

    F32 = mybir.dt.float32
    F32R = mybir.dt.float32r
    BF16 = mybir.dt.bfloat16
    AFT = mybir.ActivationFunctionType

    with (
        tc.tile_pool(name="persist", bufs=1) as pp,
        tc.tile_pool(name="wts", bufs=1) as pw,
        tc.tile_pool(name="ps5", bufs=4, space="PSUM") as ps5,
        tc.tile_pool(name="stw", bufs=2, space="PSUM") as stw,
        tc.tile_pool(name="ptmp", bufs=3) as pat,
        tc.tile_pool(name="pes", bufs=12) as pes,
        tc.tile_pool(name="pesw", bufs=12) as pesw,
        tc.tile_pool(name="pnrm", bufs=3) as pbs,
    ):
        # persistent tensors
        qkT = pp.tile([128, 12, T], BF16, tag="qkT")  # 0-5: q pairs, 6-11: k
        v_sb = pp.tile([128, NT, H, 65], BF16, tag="v")  # v natural + ones col
        yT = pp.tile([128, KC, T], BF16, tag="yT")
        xt_sb = pp.tile([128, KC, T], BF16, tag="xt")
        # packed consts: cos | signed-sin | mneg | idn | ones
        cbf = pp.tile([128, 2 * T + 320], BF16, tag="cbf")
        cf32 = pp.tile([128, 12], F32, tag="cf32")  # qk bias per pair-channel
        cos_sb = cbf[:, 0:T]
        sin_sb = cbf[:, T : 2 * T]
        mneg_sb = cbf[:, 2 * T : 2 * T + 128]
        idnr_sb = cbf[:, 2 * T + 128 : 2 * T + 256]
        ones_b = cbf[:, 2 * T + 256 : 2 * T + 320]
        bqk_sb = cf32
        bvbc_sb = pp.tile([128, C], BF16, tag="bvbc")
        bpbc_sb = pp.tile([128, C], F32, tag="bpbc")
        wp_sb = pp.tile([128, KC, C], BF16, tag="wp")

        xt_r = dr["xt"][:].rearrange("p (kc t) -> p kc t", kc=KC)

        def _load_wa(tag, g, cols):
            wt = pw.tile([128, KC, cols], BF16, tag=tag, bufs=3)
            start = KC * _WAOFF[g]
            nc.sync.dma_start(
                out=wt[:],
                in_=dr["wa"][:, start : start + KC * cols].rearrange(
                    "p (kc n) -> p kc n", kc=KC
                ),
            )
            return wt

        # --- init DMAs in first-use order (SP queue is FIFO) ---
        nc.sync.dma_start(out=xt_sb[:, :, 0:512], in_=xt_r[:, :, 0:512])
        wtp = {0: _load_wa("wtp", 2, 256)}
        nc.sync.dma_start(out=xt_sb[:, :, 512:1024], in_=xt_r[:, :, 512:1024])
        nc.sync.dma_start(out=cf32[:], in_=dr["cf32"][:])
        nc.sync.dma_start(out=cbf[:], in_=dr["cbf"][:])
        wtv = [_load_wa("wtv", 0, 384), _load_wa("wtv", 1, 384)]
        wtp[1] = _load_wa("wtp", 3, 256)
        nc.sync.dma_start(out=bvbc_sb[:], in_=dr["bvbc"][:])
        nc.sync.dma_start(out=bpbc_sb[:], in_=dr["bpbc"][:])
        # ones column of v via host DMA (DVE memset miscompiled on HW)
        nc.sync.dma_start(
            out=v_sb[:, :, :, 64:65],
            in_=dr["vones"][:].rearrange("p (a b o) -> p a b o", a=NT, b=H),
        )

        def emit_qk(i):
            wt = wtp[i]
            for which, m in ((0, i), (1, 6 + i)):
                pss = [
                    ps5.tile([128, 512], F32, tag="ps5", name=f"ps_{which}_{pj}")
                    for pj in range(2)
                ]
                if i == 0 and which == 0:
                    # startup: all pj0 matmuls first (xt half 1 still in DMA)
                    order = [(kc, pj) for pj in range(2) for kc in range(KC)]
                else:
                    # both pj share the stationary weight
                    order = [(kc, pj) for kc in range(KC) for pj in range(2)]
                for kc, pj in order:
                    nc.tensor.matmul(
                        pss[pj][:],
                        wt[:, kc, which * 128 : which * 128 + 128],
                        xt_sb[:, kc, pj * 512 : (pj + 1) * 512],
                        start=(kc == 0),
                        stop=(kc == KC - 1),
                    )
                for pj in range(2):
                    w = slice(pj * 512, (pj + 1) * 512)
                    # both q and k copies on DVE: ACT then runs only the exp
                    # stream + rs copies, finishing ~2us earlier per
                    # iteration, which un-gates the pj1 y matmuls and the
                    # normalize tail that otherwise stalls the next
                    # iteration's first score tile
                    nc.vector.tensor_scalar_add(
                        qkT[:, m, w], pss[pj][:], bqk_sb[:, m : m + 1]
                    )

        # rope rotation = adjacent-partition swap; the sign lives in the
        # host-packed signed sin table, so no PE rotation matmul is needed.
        # (GPSIMD is avoided: its tensor ops produce garbage on the first
        # post-load execution.)
        swap_mask = [i ^ 1 for i in range(32)]

        def emit_rope(i):
            for m in (i, 6 + i):
                shf = pat.tile([128, T], BF16, tag="shf", bufs=2)
                nc.vector.stream_shuffle(shf[:], qkT[:, m, :], swap_mask)
                t1 = pat.tile([128, T], BF16, tag="t1", bufs=2)
                nc.vector.tensor_mul(t1[:], qkT[:, m, :], cos_sb[:])
                t2 = pat.tile([128, T], BF16, tag="t2", bufs=2)
                nc.vector.tensor_mul(t2[:], shf[:], sin_sb[:])
                nc.vector.tensor_add(qkT[:, m, :], t1[:], t2[:])

        es_store = {}

        def emit_s(i, tkcs):
            qv, kv = i, 6 + i
            for tkc in tkcs:
                lo = 128 * tkc
                width = T - lo
                wide = width > 512
                for hh in range(2):
                    b0 = 64 * hh
                    if wide:
                        st = stw.tile([128, 1024], F32, tag="stw")
                    else:
                        st = ps5.tile([128, 512], F32, tag="ps5")
                    for off in range(0, width, 512):
                        valid_w = min(512, width - off)
                        nc.tensor.matmul(
                            st[:, off : off + valid_w],
                            qkT[b0 : b0 + 64, kv, lo : lo + 128],
                            qkT[b0 : b0 + 64, qv, lo + off : lo + off + valid_w],
                            start=True,
                            stop=not (off == 0),
                        )
                        if off == 0:  # causal mask add on the diagonal block
                            nc.tensor.matmul(
                                st[:, 0:128],
                                idnr_sb,
                                mneg_sb,
                                start=False,
                                stop=True,
                            )
                    if wide:
                        es = pesw.tile([128, 1024], BF16, tag="esw", name="esw")
                    else:
                        es = pes.tile([128, 512], BF16, tag="es", name="es")
                    nc.scalar.activation(
                        es[:, :width], st[:, :width], AFT.Exp, scale=0.125
                    )
                    es_store[(hh, tkc)] = es

        def emit_v():
            for vg in range(2):
                wt = wtv[vg]
                for t in range(NT):
                    ps = ps5.tile([128, 512], F32, tag="ps5")
                    for kc in range(KC):
                        nc.tensor.matmul(
                            ps[:, 0:384],
                            xt_sb[:, kc, t * 128 : (t + 1) * 128],
                            wt[:, kc, :],
                            start=(kc == 0),
                            stop=(kc == KC - 1),
                        )
                    nc.vector.tensor_add(
                        v_sb[:, t, 6 * vg : 6 * vg + 6, 0:64],
                        ps[:, 0:384].rearrange("p (h d) -> p h d", h=6),
                        bvbc_sb[:, vg * 384 : (vg + 1) * 384].rearrange(
                            "p (h d) -> p h d", h=6
                        ),
                    )

        def emit_y(i, pjs=(0, 1)):
            for pj in pjs:
                w0 = 512 * pj
                tkcs = [k for k in range(NT) if 128 * k < w0 + 512]
                for hh in range(2):
                    h = 2 * i + hh
                    yp = ps5.tile([128, 512], F32, tag="ps5")
                    for j, tkc in enumerate(tkcs):
                        lo = 128 * tkc
                        plo = max(w0, lo)
                        wdt = w0 + 512 - plo
                        es = es_store[(hh, tkc)]
                        nc.tensor.matmul(
                            yp[0:65, plo - w0 : plo - w0 + wdt],
                            v_sb[:, tkc, h, :],
                            es[:, plo - lo : plo - lo + wdt],
                            start=(j == 0),
                            stop=(j == len(tkcs) - 1),
                        )
                    # normalize: y = y' / r  (r = row 64 of yp):
                    # r -> SBUF f32r, broadcast to 64 rows on PE, reciprocal
                    # moves it back to SBUF, multiply
                    rs = pbs.tile([128, 512], BF16, tag="rs")
                    nc.scalar.activation(
                        rs[64:65, :], yp[64:65, :], AFT.Identity
                    )
                    rbp = stw.tile([128, 1024], F32, tag="stw")
                    nc.tensor.matmul(
                        rbp[0:64, 0:512],
                        ones_b[64:65, :],
                        rs[64:65, :],
                        start=True,
                        stop=True,
                    )
                    rbf = pbs.tile([64, 512], F32, tag="rbf")
                    nc.vector.reciprocal_approx_fast(
                        out=rbf[:], in_=rbp[0:64, 0:512]
                    )
                    if hh == 0:
                        nc.vector.tensor_mul(
                            yT[0:64, i, w0 : w0 + 512], yp[0:64, :], rbf[:]
                        )
                    else:
                        ys = pbs.tile([64, 512], BF16, tag="ys")
                        nc.vector.tensor_mul(ys[:], yp[0:64, :], rbf[:])
                        nc.sync.dma_start(
                            out=yT[64:128, i, w0 : w0 + 512], in_=ys[:]
                        )

        # ---------------- pipelined qkv + attention ----------------
        if 1 not in phases:
            return
        emit_qk(0)
        emit_rope(0)
        for i in range(1, 7):
            if i + 1 < 6:
                wtp[i + 1] = _load_wa("wtp", 2 + (i + 1), 256)
            if i == 4:
                nc.sync.dma_start(
                    out=wp_sb[:],
                    in_=dr["wp"][:].rearrange("p (kc n) -> p kc n", kc=KC),
                )
            if 2 in phases:
                emit_s(i - 1, range(0, 2))
            if i == 1:
                emit_v()
            if i < 6:
                emit_qk(i)
                emit_rope(i)
            if 2 in phases:
                emit_s(i - 1, range(2, NT))
                emit_y(i - 1)

        if _DEBUG:
            nc.sync.dma_start(
                out=dr["dqk"][:], in_=qkT[:].rearrange("p a b -> p (a b)")
            )
            nc.sync.dma_start(
                out=dr["dv"][:], in_=v_sb[:].rearrange("p a b c -> p (a b c)")
            )
            nc.sync.dma_start(
                out=dr["dyt"][:], in_=yT[:].rearrange("p a b -> p (a b)")
            )

        # ---------------- output projection ----------------
        if 3 not in phases:
            return
        with tc.tile_pool(name="pc_ob", bufs=3) as pco:
            for m in range(NT):
                osb = pco.tile([128, C], F32, tag="ob")
                for piece in range(2):
                    pw_ = slice(piece * 384, (piece + 1) * 384)
                    po = ps5.tile([128, 512], F32, tag="ps5")
                    for kc in range(KC):
                        nc.tensor.matmul(
                            po[:, 0:384],
                            yT[:, kc, m * 128 : (m + 1) * 128],
                            wp_sb[:, kc, pw_],
                            start=(kc == 0),
                            stop=(kc == KC - 1),
                        )
                    nc.vector.tensor_add(osb[:, pw_], po[:, 0:384], bpbc_sb[:, pw_])
                    nc.sync.dma_start(
                        out=dr["out"][m * 128 : (m + 1) * 128, pw_],
                        in_=osb[:, pw_],
                    )


def _build_program(loop_n=None, phases=(1, 2, 3)):
    import concourse.bacc as bacc
    import concourse.tile as tile
    from concourse import mybir

    F32 = mybir.dt.float32
    BF16 = mybir.dt.bfloat16

    nc = bacc.Bacc(None, target_bir_lowering=False, debug=False)

    dr = {
        "xt": nc.dram_tensor("xt", [128, KC * T], BF16, kind="ExternalInput"),
        "wa": nc.dram_tensor("wa", [128, KC * _WATOT], BF16, kind="ExternalInput"),
        "bvbc": nc.dram_tensor("bvbc", [128, C], BF16, kind="ExternalInput"),
        "wp": nc.dram_tensor("wp", [128, KC * C], BF16, kind="ExternalInput"),
        "bpbc": nc.dram_tensor("bpbc", [128, C], F32, kind="ExternalInput"),
        "cbf": nc.dram_tensor("cbf", [128, 2 * T + 320], BF16, kind="ExternalInput"),
        "cf32": nc.dram_tensor("cf32", [128, 12], F32, kind="ExternalInput"),
        "vones": nc.dram_tensor("vones", [128, NT * H], BF16, kind="ExternalInput"),
        "out": nc.dram_tensor("out", [T, C], F32, kind="ExternalOutput"),
    }
    if _DEBUG:
        dr["dqk"] = nc.dram_tensor("dqk", [128, 12 * T], BF16, kind="ExternalOutput")
        dr["dv"] = nc.dram_tensor("dv", [128, NT * H * 65], BF16, kind="ExternalOutput")
        dr["dyt"] = nc.dram_tensor("dyt", [128, KC * T], BF16, kind="ExternalOutput")

    with tile.TileContext(nc) as tc:
        if loop_n is None:
            _emit_body(nc, tc, dr, phases)
        else:
            with tc.For_i(0, loop_n, 1):
                _emit_body(nc, tc, dr, phases)

    nc.compile()
    return nc


def _host_constants():
    """Constant tables shipped to every core."""
    inv_freq = (1.0 / (10000.0 ** (np.arange(0, D, 2, dtype=np.float32) / D))).astype(
        np.float32
    )
    tpos = np.arange(T, dtype=np.float32)
    freqs = tpos[None, :] * inv_freq[:, None]  # [32, T]
    cos32 = np.cos(freqs).astype(np.float32)
    sin32 = np.sin(freqs).astype(np.float32)
    cosT = np.repeat(cos32, 2, axis=0)  # [64, T], channel d -> freq d//2
    sinT = np.repeat(sin32, 2, axis=0)
    cosT = np.concatenate([cosT, cosT], axis=0)  # [128, T]: two head copies
    sinT = np.concatenate([sinT, sinT], axis=0)

    # rotation = adjacent-row swap; fold the signs into the sin table:
    # rot[2i] = -q[2i+1]*sin, rot[2i+1] = +q[2i]*sin
    sinS = sinT.copy()
    sinS[0::2, :] *= -1.0

    import ml_dtypes

    # additive mask in s^T orientation: -1e5 (pre-scale) where tq_rel < tk
    mneg = (-1.0e5 * np.tril(np.ones((128, 128), dtype=np.float32), k=-1)).astype(
        ml_dtypes.bfloat16
    )
    idn_b = np.eye(128, dtype=np.float32).astype(ml_dtypes.bfloat16)
    return cosT, sinS, mneg, idn_b


def _input_maps(x, W_attn, b_attn, W_proj, b_proj):
    import ml_dtypes

    BF = ml_dtypes.bfloat16
    cosT, sinS, mneg_b, idn_b = _host_constants()

    # wa: [C, 3C] -> [128, kc, n], columns packed as [vA | vB | pair0..pair5]
    wa = W_attn.reshape(KC, 128, 3 * C).transpose(1, 0, 2)  # [128, kc, 3C]
    groups = [wa[:, :, 2 * C : 2 * C + 384], wa[:, :, 2 * C + 384 : 3 * C]]
    for hp in range(6):
        groups.append(wa[:, :, hp * 128 : (hp + 1) * 128])  # q pair
        groups.append(wa[:, :, C + hp * 128 : C + (hp + 1) * 128])  # k pair
    # merge each pair's q+k into one 256-col group
    packed = [groups[0], groups[1]] + [
        np.concatenate([groups[2 + 2 * hp], groups[3 + 2 * hp]], axis=2)
        for hp in range(6)
    ]
    wa_g = np.concatenate([g.reshape(128, -1) for g in packed], axis=1)
    assert wa_g.shape[1] == KC * _WATOT

    wp = W_proj.reshape(KC, 128, C).transpose(1, 0, 2).reshape(128, KC * C)

    bqk = b_attn[: 2 * C].reshape(12, 128).T.astype(np.float32)
    cbf = np.concatenate(
        [
            cosT.astype(BF),
            sinS.astype(BF),
            mneg_b,
            idn_b,
            np.ones((128, 64), BF),
        ],
        axis=1,
    )
    cf32 = np.ascontiguousarray(bqk)
    shared = {
        "wa": np.ascontiguousarray(wa_g.astype(BF)),
        "bvbc": np.ascontiguousarray(
            np.broadcast_to(b_attn[2 * C :].astype(BF), (128, C))
        ),
        "wp": np.ascontiguousarray(wp.astype(BF)),
        "bpbc": np.ascontiguousarray(np.broadcast_to(b_proj, (128, C))),
        "cbf": np.ascontiguousarray(cbf),
        "cf32": cf32,
        "vones": np.ones((128, NT * H), dtype=BF),
    }
    out = []
    for b in range(B):
        xt = (
            x[b].T.reshape(KC, 128, T).transpose(1, 0, 2).reshape(128, KC * T)
        )  # [128, KC*T]
        out.append(dict(shared, xt=np.ascontiguousarray(xt.astype(BF))))
    return out


def kernel(x, W_attn, b_attn, W_proj, b_proj):
    global _prog
    from concourse.bass_utils import run_bass_kernel_spmd

    if _prog is None:
        _prog = _build_program()

    x = np.asarray(x, dtype=np.float32)
    W_attn = np.asarray(W_attn, dtype=np.float32)
    b_attn = np.asarray(b_attn, dtype=np.float32)
    W_proj = np.asarray(W_proj, dtype=np.float32)
    b_proj = np.asarray(b_proj, dtype=np.float32)

    in_maps = _input_maps(x, W_attn, b_attn, W_proj, b_proj)
    # first post-load execution shows cold-start wobble in some ucode
    # engines; run once to warm up, return the steady-state result
    # retry guard: transient device errors (NRT unrecoverable) have been
    # observed on this fleet; a failed attempt costs only wall-clock
    res = None
    for attempt in range(3):
        try:
            run_bass_kernel_spmd(_prog, in_maps, list(range(N_CORES)))
            res = run_bass_kernel_spmd(_prog, in_maps, list(range(N_CORES)))
            break
        except Exception:
            if attempt == 2:
                raise
    out = np.stack([res.results[b]["out"] for b in range(B)], axis=0)
    return out.astype(np.float32)



# revision 9
# speedup vs baseline: 1.3376x; 1.3376x over previous
"""Causal self-attention with RoPE on 8 TRN2 NeuronCores.

Sharding: pure data parallel over batch B=8 (one batch element per core,
weights replicated, no collectives).

Per-core dataflow, all matmuls bf16 with fp32 PSUM accumulation. The host
pre-transposes x, pre-packs W_attn by head-pair, pre-broadcasts biases and
pre-signs the sin table.

  xT (host transpose)                DMA                       [C, T]
  q^T,k^T = W_qk^T @ x + b           PE (W stationary); psum drain with
                                     fused bias on GPSIMD/Pool
  v natural = x @ W_v + b            PE; Pool bias-add         [T, ch]
  RoPE(q,k) in place                 DVE only: stream_shuffle pair-swap,
                                     q*cos + swap(q)*signed_sin
  s^T = k @ q^T (per head)           PE K=64, <=512-wide psum  [Tk, Tq]
  p = exp(s/8)                       ACT exp; causal mask = Pool multiply
                                     by triu-ones on the diagonal block
  [y | r] = p^T @ [v, 1]  (natural)  PE K=128 accum            [Tq, 65]
  y = y * (1/r)                      DVE reciprocal [128,1] -> Pool
                                     tensor_scalar_mul (per-partition r)
  yT via DMA-engine transpose        XBAR dma_start_transpose  [C, T]
  out = y @ W_proj + b               PE; Pool bias-add         [T, C] f32

PSUM (8 banks): pb 3x[128,512] (qkv/v/proj), psc 3x[128,512] (scores),
py 2x[128,512] (y accumulators, col 64 = softmax denominator r).

HW notes kept from earlier sessions: kernel() runs the NEFF twice and
returns the steady-state result (first post-load execution wobble); DVE
memsets miscompile on this toolchain (constants arrive via host DMAs).
"""
import sys

sys.path.insert(0, "/opt/trn_rl_repo")

import numpy as np

B, T, C = 8, 1024, 768
H, D = 12, 64
N_CORES = 8
KC = C // 128  # 6 K-chunks of the C contraction
NT = T // 128  # 8 T-chunks

# wa group offsets (in columns of the host-packed [128, KC, ...] layout):
# group 0: vA (384 cols), group 1: vB (384), groups 2..7: pair hp (256)
_WAOFF = [0, 384, 768, 1024, 1280, 1536, 1792, 2048]  # start col of each group
_WATOT = 2304  # total packed columns

_prog = None  # cached compiled Bass program
_DEBUG = False  # add intermediate-dump DMAs (qkT, v_sb, yT)


def _emit_body(nc, tc, dr, phases=(1, 2, 3)):
    """Emit one full forward pass. dr = dict of DRAM tensors."""
    from concourse import mybir

    F32 = mybir.dt.float32
    BF16 = mybir.dt.bfloat16
    AFT = mybir.ActivationFunctionType

    with (
        tc.tile_pool(name="persist", bufs=1) as pp,
        tc.tile_pool(name="wts", bufs=1) as pw,
        tc.tile_pool(name="pb", bufs=2, space="PSUM") as pb,
        tc.tile_pool(name="psc", bufs=2, space="PSUM") as psc,
        tc.tile_pool(name="py", bufs=2, space="PSUM") as py,
        tc.tile_pool(name="ptmp", bufs=3) as pat,
        tc.tile_pool(name="pes", bufs=10) as pes,
        tc.tile_pool(name="pesn", bufs=10) as pesn,
        tc.tile_pool(name="pnrm", bufs=6) as pbs,
    ):
        # persistent tensors
        qkT = pp.tile([128, 12, T], BF16, tag="qkT")  # 0-5: q pairs, 6-11: k
        v_sb = pp.tile([128, NT, H, 65], BF16, tag="v")  # v natural + ones col
        yT = pp.tile([128, KC, T], BF16, tag="yT")
        ynat = pp.tile([128, NT, C], BF16, tag="ynat")  # normalized y, natural
        xt_sb = pp.tile([128, KC, T], BF16, tag="xt")
        # packed consts: cos | signed-sin | triu-ones (causal keep mask)
        cbf = pp.tile([128, 2 * T + 128], BF16, tag="cbf")
        cf32 = pp.tile([128, 12], F32, tag="cf32")  # qk bias per pair-channel
        cos_sb = cbf[:, 0:T]
        sin_sb = cbf[:, T : 2 * T]
        tri_sb = cbf[:, 2 * T : 2 * T + 128]
        bqk_sb = cf32
        bvbc_sb = pp.tile([128, C], BF16, tag="bvbc")
        bpbc_sb = pp.tile([128, C], F32, tag="bpbc")
        wp_sb = pp.tile([128, KC, C], BF16, tag="wp")

        xt_r = dr["xt"][:].rearrange("p (kc t) -> p kc t", kc=KC)

        def _load_wa(tag, g, cols):
            wt = pw.tile([128, KC, cols], BF16, tag=tag, bufs=3)
            start = KC * _WAOFF[g]
            nc.sync.dma_start(
                out=wt[:],
                in_=dr["wa"][:, start : start + KC * cols].rearrange(
                    "p (kc n) -> p kc n", kc=KC
                ),
            )
            return wt

        # --- init DMAs in first-use order (SP queue is FIFO) ---
        nc.sync.dma_start(out=xt_sb[:, :, 0:512], in_=xt_r[:, :, 0:512])
        wtp = {0: _load_wa("wtp", 2, 256)}
        nc.sync.dma_start(out=xt_sb[:, :, 512:1024], in_=xt_r[:, :, 512:1024])
        nc.sync.dma_start(out=cf32[:], in_=dr["cf32"][:])
        nc.sync.dma_start(out=cbf[:], in_=dr["cbf"][:])
        wtv = [_load_wa("wtv", 0, 384), _load_wa("wtv", 1, 384)]
        wtp[1] = _load_wa("wtp", 3, 256)
        nc.sync.dma_start(out=bvbc_sb[:], in_=dr["bvbc"][:])
        nc.sync.dma_start(out=bpbc_sb[:], in_=dr["bpbc"][:])
        # ones column of v via host DMA (DVE memset miscompiled on HW)
        nc.sync.dma_start(
            out=v_sb[:, :, :, 64:65],
            in_=dr["vones"][:].rearrange("p (a b o) -> p a b o", a=NT, b=H),
        )

        def emit_qk(i):
            wt = wtp[i]
            for which, m in ((0, i), (1, 6 + i)):
                pss = [
                    pb.tile([128, 512], F32, tag="pb", name=f"ps_{which}_{pj}")
                    for pj in range(2)
                ]
                if i == 0 and which == 0:
                    # startup: all pj0 matmuls first (xt half 1 still in DMA)
                    order = [(kc, pj) for pj in range(2) for kc in range(KC)]
                else:
                    # both pj share the stationary weight
                    order = [(kc, pj) for kc in range(KC) for pj in range(2)]
                for kc, pj in order:
                    nc.tensor.matmul(
                        pss[pj][:],
                        wt[:, kc, which * 128 : which * 128 + 128],
                        xt_sb[:, kc, pj * 512 : (pj + 1) * 512],
                        start=(kc == 0),
                        stop=(kc == KC - 1),
                    )
                for pj in range(2):
                    w = slice(pj * 512, (pj + 1) * 512)
                    # psum drain + fused bias (GPSIMD cannot access PSUM)
                    nc.vector.tensor_scalar_add(
                        qkT[:, m, w], pss[pj][:], bqk_sb[:, m : m + 1]
                    )

        # rope rotation = adjacent-partition swap; the sign lives in the
        # host-packed signed sin table.
        swap_mask = [i ^ 1 for i in range(32)]

        def emit_rope(i):
            # shuffle on DVE (only engine with stream_shuffle); the three
            # elementwise ops run on the otherwise-idle Pool engine (all-SBUF
            # operands, so GPSIMD is legal here)
            for m in (i, 6 + i):
                shf = pat.tile([128, T], BF16, tag="shf", bufs=2)
                nc.vector.stream_shuffle(shf[:], qkT[:, m, :], swap_mask)
                t1 = pat.tile([128, T], BF16, tag="t1", bufs=2)
                nc.gpsimd.tensor_mul(t1[:], qkT[:, m, :], cos_sb[:])
                t2 = pat.tile([128, T], BF16, tag="t2", bufs=2)
                nc.gpsimd.tensor_mul(t2[:], shf[:], sin_sb[:])
                nc.gpsimd.tensor_add(qkT[:, m, :], t1[:], t2[:])

        es_store = {}

        def emit_s(i, tkcs):
            qv, kv = i, 6 + i
            for tkc in tkcs:
                lo = 128 * tkc
                width = T - lo
                for hh in range(2):
                    b0 = 64 * hh
                    st = psc.tile([128, 1024], F32, tag="psc")
                    for off in range(0, width, 512):
                        w = min(512, width - off)
                        nc.tensor.matmul(
                            st[:, off : off + w],
                            qkT[b0 : b0 + 64, kv, lo : lo + 128],
                            qkT[b0 : b0 + 64, qv, lo + off : lo + off + w],
                            start=True,
                            stop=True,
                        )
                    if width > 512:
                        es = pes.tile([128, 1024], BF16, tag="esw")
                    else:
                        es = pesn.tile([128, 512], BF16, tag="esn")
                    nc.scalar.activation(
                        es[:, 0:width], st[:, 0:width], AFT.Exp, scale=0.125
                    )
                    # causal mask: zero the upper triangle (tk > tq) of the
                    # diagonal block, in place, on Pool (all-SBUF: legal)
                    nc.gpsimd.tensor_mul(es[:, 0:128], es[:, 0:128], tri_sb)
                    es_store[(hh, tkc)] = es

        def emit_v():
            for vg in range(2):
                wt = wtv[vg]
                for t in range(NT):
                    ps = pb.tile([128, 512], F32, tag="pb")
                    for kc in range(KC):
                        nc.tensor.matmul(
                            ps[:, 0:384],
                            xt_sb[:, kc, t * 128 : (t + 1) * 128],
                            wt[:, kc, :],
                            start=(kc == 0),
                            stop=(kc == KC - 1),
                        )
                    nc.vector.tensor_add(
                        v_sb[:, t, 6 * vg : 6 * vg + 6, 0:64],
                        ps[:, 0:384].rearrange("p (h d) -> p h d", h=6),
                        bvbc_sb[:, vg * 384 : (vg + 1) * 384].rearrange(
                            "p (h d) -> p h d", h=6
                        ),
                    )

        def emit_proj_m(m):
            for piece in range(2):
                pw_ = slice(piece * 384, (piece + 1) * 384)
                po = pb.tile([128, 512], F32, tag="pb")
                for kc in range(KC):
                    nc.tensor.matmul(
                        po[:, 0:384],
                        yT[:, kc, m * 128 : (m + 1) * 128],
                        wp_sb[:, kc, pw_],
                        start=(kc == 0),
                        stop=(kc == KC - 1),
                    )
                osb = pbs.tile([128, 384], F32, tag="ob", bufs=3)
                nc.vector.tensor_add(osb[:], po[:, 0:384], bpbc_sb[:, pw_])
                nc.sync.dma_start(
                    out=dr["out"][m * 128 : (m + 1) * 128, pw_], in_=osb[:]
                )

        def emit_y(i, final):
            for qb in range(NT):
                for hh in range(2):
                    h = 2 * i + hh
                    yp = py.tile([128, 512], F32, tag="py")
                    for j, tkc in enumerate(range(qb + 1)):
                        rel = qb * 128 - 128 * tkc
                        es = es_store[(hh, tkc)]
                        nc.tensor.matmul(
                            yp[:, 0:65],
                            es[:, rel : rel + 128],
                            v_sb[:, tkc, h, :],
                            start=(j == 0),
                            stop=(j == qb),
                        )
                    rinv = pbs.tile([128, 1], F32, tag="rinv", bufs=6)
                    nc.vector.reciprocal_approx_fast(
                        out=rinv[:], in_=yp[:, 64:65]
                    )
                    nc.vector.tensor_scalar_mul(
                        ynat[:, qb, h * 64 : (h + 1) * 64], yp[:, 0:64], rinv[:]
                    )
                # channels of pair i are exactly contraction chunk kc=i:
                # transpose this pair's [tq, 128ch] block on the DMA xbar
                nc.sync.dma_start_transpose(
                    out=yT[:, i, qb * 128 : (qb + 1) * 128],
                    in_=ynat[:, qb, i * 128 : (i + 1) * 128],
                )
                if final and 3 in phases and qb >= 2:
                    emit_proj_m(qb - 2)
            if final and 3 in phases:
                emit_proj_m(NT - 2)
                emit_proj_m(NT - 1)

        # ---------------- pipelined qkv + attention ----------------
        if 1 not in phases:
            return
        emit_qk(0)
        emit_rope(0)
        for i in range(1, 7):
            if i + 1 < 6:
                wtp[i + 1] = _load_wa("wtp", 2 + (i + 1), 256)
            if i == 4:
                nc.sync.dma_start(
                    out=wp_sb[:],
                    in_=dr["wp"][:].rearrange("p (kc n) -> p kc n", kc=KC),
                )
            if 2 in phases:
                emit_s(i - 1, range(0, 2))
            if i == 1:
                emit_v()
            if i < 6:
                emit_qk(i)
                emit_rope(i)
            if 2 in phases:
                emit_s(i - 1, range(2, NT))
                emit_y(i - 1, final=(i == 6))

        if _DEBUG:
            nc.sync.dma_start(
                out=dr["dqk"][:], in_=qkT[:].rearrange("p a b -> p (a b)")
            )
            nc.sync.dma_start(
                out=dr["dv"][:], in_=v_sb[:].rearrange("p a b c -> p (a b c)")
            )
            nc.sync.dma_start(
                out=dr["dyt"][:], in_=yT[:].rearrange("p a b -> p (a b)")
            )


def _build_program(loop_n=None, phases=(1, 2, 3)):
    import concourse.bacc as bacc
    import concourse.tile as tile
    from concourse import mybir

    F32 = mybir.dt.float32
    BF16 = mybir.dt.bfloat16

    nc = bacc.Bacc(None, target_bir_lowering=False, debug=False)

    dr = {
        "xt": nc.dram_tensor("xt", [128, KC * T], BF16, kind="ExternalInput"),
        "wa": nc.dram_tensor("wa", [128, KC * _WATOT], BF16, kind="ExternalInput"),
        "bvbc": nc.dram_tensor("bvbc", [128, C], BF16, kind="ExternalInput"),
        "wp": nc.dram_tensor("wp", [128, KC * C], BF16, kind="ExternalInput"),
        "bpbc": nc.dram_tensor("bpbc", [128, C], F32, kind="ExternalInput"),
        "cbf": nc.dram_tensor("cbf", [128, 2 * T + 128], BF16, kind="ExternalInput"),
        "cf32": nc.dram_tensor("cf32", [128, 12], F32, kind="ExternalInput"),
        "vones": nc.dram_tensor("vones", [128, NT * H], BF16, kind="ExternalInput"),
        "out": nc.dram_tensor("out", [T, C], F32, kind="ExternalOutput"),
    }
    if _DEBUG:
        dr["dqk"] = nc.dram_tensor("dqk", [128, 12 * T], BF16, kind="ExternalOutput")
        dr["dv"] = nc.dram_tensor("dv", [128, NT * H * 65], BF16, kind="ExternalOutput")
        dr["dyt"] = nc.dram_tensor("dyt", [128, KC * T], BF16, kind="ExternalOutput")

    with tile.TileContext(nc) as tc:
        if loop_n is None:
            _emit_body(nc, tc, dr, phases)
        else:
            with tc.For_i(0, loop_n, 1):
                _emit_body(nc, tc, dr, phases)

    nc.compile()
    return nc


def _host_constants():
    """Constant tables shipped to every core."""
    inv_freq = (1.0 / (10000.0 ** (np.arange(0, D, 2, dtype=np.float32) / D))).astype(
        np.float32
    )
    tpos = np.arange(T, dtype=np.float32)
    freqs = tpos[None, :] * inv_freq[:, None]  # [32, T]
    cos32 = np.cos(freqs).astype(np.float32)
    sin32 = np.sin(freqs).astype(np.float32)
    cosT = np.repeat(cos32, 2, axis=0)  # [64, T], channel d -> freq d//2
    sinT = np.repeat(sin32, 2, axis=0)
    cosT = np.concatenate([cosT, cosT], axis=0)  # [128, T]: two head copies
    sinT = np.concatenate([sinT, sinT], axis=0)

    # rotation = adjacent-row swap; fold the signs into the sin table:
    # rot[2i] = -q[2i+1]*sin, rot[2i+1] = +q[2i]*sin
    sinS = sinT.copy()
    sinS[0::2, :] *= -1.0

    import ml_dtypes

    # multiplicative keep-mask in s^T orientation: keep tk <= tq
    tri = np.triu(np.ones((128, 128), dtype=np.float32)).astype(ml_dtypes.bfloat16)
    return cosT, sinS, tri


def _input_maps(x, W_attn, b_attn, W_proj, b_proj):
    import ml_dtypes

    BF = ml_dtypes.bfloat16
    cosT, sinS, tri_b = _host_constants()

    # wa: [C, 3C] -> [128, kc, n], columns packed as [vA | vB | pair0..pair5]
    wa = W_attn.reshape(KC, 128, 3 * C).transpose(1, 0, 2)  # [128, kc, 3C]
    groups = [wa[:, :, 2 * C : 2 * C + 384], wa[:, :, 2 * C + 384 : 3 * C]]
    for hp in range(6):
        groups.append(wa[:, :, hp * 128 : (hp + 1) * 128])  # q pair
        groups.append(wa[:, :, C + hp * 128 : C + (hp + 1) * 128])  # k pair
    # merge each pair's q+k into one 256-col group
    packed = [groups[0], groups[1]] + [
        np.concatenate([groups[2 + 2 * hp], groups[3 + 2 * hp]], axis=2)
        for hp in range(6)
    ]
    wa_g = np.concatenate([g.reshape(128, -1) for g in packed], axis=1)
    assert wa_g.shape[1] == KC * _WATOT

    wp = W_proj.reshape(KC, 128, C).transpose(1, 0, 2).reshape(128, KC * C)

    bqk = b_attn[: 2 * C].reshape(12, 128).T.astype(np.float32)
    cbf = np.concatenate(
        [cosT.astype(BF), sinS.astype(BF), tri_b],
        axis=1,
    )
    cf32 = np.ascontiguousarray(bqk)
    shared = {
        "wa": np.ascontiguousarray(wa_g.astype(BF)),
        "bvbc": np.ascontiguousarray(
            np.broadcast_to(b_attn[2 * C :].astype(BF), (128, C))
        ),
        "wp": np.ascontiguousarray(wp.astype(BF)),
        "bpbc": np.ascontiguousarray(np.broadcast_to(b_proj, (128, C))),
        "cbf": np.ascontiguousarray(cbf),
        "cf32": cf32,
        "vones": np.ones((128, NT * H), dtype=BF),
    }
    out = []
    for b in range(B):
        xt = (
            x[b].T.reshape(KC, 128, T).transpose(1, 0, 2).reshape(128, KC * T)
        )  # [128, KC*T]
        out.append(dict(shared, xt=np.ascontiguousarray(xt.astype(BF))))
    return out


def kernel(x, W_attn, b_attn, W_proj, b_proj):
    global _prog
    from concourse.bass_utils import run_bass_kernel_spmd

    if _prog is None:
        _prog = _build_program()

    x = np.asarray(x, dtype=np.float32)
    W_attn = np.asarray(W_attn, dtype=np.float32)
    b_attn = np.asarray(b_attn, dtype=np.float32)
    W_proj = np.asarray(W_proj, dtype=np.float32)
    b_proj = np.asarray(b_proj, dtype=np.float32)

    in_maps = _input_maps(x, W_attn, b_attn, W_proj, b_proj)
    # first post-load execution shows cold-start wobble in some ucode
    # engines; run once to warm up, return the steady-state result
    # retry guard: transient device errors (NRT unrecoverable) have been
    # observed on this fleet; a failed attempt costs only wall-clock
    res = None
    for attempt in range(3):
        try:
            run_bass_kernel_spmd(_prog, in_maps, list(range(N_CORES)))
            res = run_bass_kernel_spmd(_prog, in_maps, list(range(N_CORES)))
            break
        except Exception:
            if attempt == 2:
                raise
    out = np.stack([res.results[b]["out"] for b in range(B)], axis=0)
    return out.astype(np.float32)


# revision 22
# speedup vs baseline: 1.4091x; 1.0535x over previous
"""Causal self-attention with RoPE on 8 TRN2 NeuronCores.

Sharding: pure data parallel over batch B=8 (one batch element per core,
weights replicated, no collectives).

Per-core dataflow, all matmuls bf16 with fp32 PSUM accumulation. The host
pre-transposes x, pre-packs W_attn by head-pair, pre-broadcasts biases and
pre-signs the sin table.

  xT (host transpose)                DMA                       [C, T]
  q^T,k^T = W_qk^T @ x + b           PE (W stationary); psum drain with
                                     fused bias on GPSIMD/Pool
  v natural = x @ W_v + b            PE; Pool bias-add         [T, ch]
  RoPE(q,k) in place                 DVE only: stream_shuffle pair-swap,
                                     q*cos + swap(q)*signed_sin
  s^T = k @ q^T (per head)           PE K=64, <=512-wide psum  [Tk, Tq]
  p = exp(s/8)                       ACT exp; causal mask = Pool multiply
                                     by triu-ones on the diagonal block
  [y | r] = p^T @ [v, 1]  (natural)  PE K=128 accum            [Tq, 65]
  y = y * (1/r)                      DVE reciprocal [128,1] -> Pool
                                     tensor_scalar_mul (per-partition r)
  yT via DMA-engine transpose        XBAR dma_start_transpose  [C, T]
  out = y @ W_proj + b               PE; Pool bias-add         [T, C] f32

PSUM (8 banks): pb 3x[128,512] (qkv/v/proj), psc 3x[128,512] (scores),
py 2x[128,512] (y accumulators, col 64 = softmax denominator r).

HW notes kept from earlier sessions: kernel() runs the NEFF twice and
returns the steady-state result (first post-load execution wobble); DVE
memsets miscompile on this toolchain (constants arrive via host DMAs).
"""
import sys

sys.path.insert(0, "/opt/trn_rl_repo")

import numpy as np

B, T, C = 8, 1024, 768
H, D = 12, 64
N_CORES = 8
KC = C // 128  # 6 K-chunks of the C contraction
NT = T // 128  # 8 T-chunks

# wa group offsets (in columns of the host-packed [128, KC, ...] layout):
# group 0: vA (384 cols), group 1: vB (384), groups 2..7: pair hp (256)
_WAOFF = [0, 384, 768, 1024, 1280, 1536, 1792, 2048]  # start col of each group
_WATOT = 2304  # total packed columns

_prog = None  # cached compiled Bass program
_DEBUG = False  # add intermediate-dump DMAs (qkT, v_sb, yT)


def _emit_body(nc, tc, dr, phases=(1, 2, 3)):
    """Emit one full forward pass. dr = dict of DRAM tensors."""
    from concourse import mybir

    F32 = mybir.dt.float32
    BF16 = mybir.dt.bfloat16
    AFT = mybir.ActivationFunctionType

    with (
        tc.tile_pool(name="persist", bufs=1) as pp,
        tc.tile_pool(name="wts", bufs=1) as pw,
        tc.tile_pool(name="pb", bufs=2, space="PSUM") as pb,
        tc.tile_pool(name="psc", bufs=2, space="PSUM") as psc,
        tc.tile_pool(name="py", bufs=2, space="PSUM") as py,
        tc.tile_pool(name="ptmp", bufs=3) as pat,
        tc.tile_pool(name="pes", bufs=10) as pes,
        tc.tile_pool(name="pesn", bufs=10) as pesn,
        tc.tile_pool(name="pnrm", bufs=6) as pbs,
    ):
        # persistent tensors
        qkT = pp.tile([128, 12, T], BF16, tag="qkT")  # 0-5: q pairs, 6-11: k
        v_sb = pp.tile([128, NT, H, 65], BF16, tag="v")  # v natural + ones col
        yT = pp.tile([128, KC, T], BF16, tag="yT")
        ynat = pp.tile([128, NT, C], BF16, tag="ynat")  # normalized y, natural
        xt_sb = pp.tile([128, KC, T], BF16, tag="xt")
        # packed consts: cos | signed-sin | triu-ones (causal keep mask)
        cbf = pp.tile([128, 2 * T + 128], BF16, tag="cbf")
        cf32 = pp.tile([128, 12], F32, tag="cf32")  # qk bias per pair-channel
        cos_sb = cbf[:, 0:T]
        sin_sb = cbf[:, T : 2 * T]
        tri_sb = cbf[:, 2 * T : 2 * T + 128]
        bqk_sb = cf32
        bvbc_sb = pp.tile([128, C], BF16, tag="bvbc")
        bpbc_sb = pp.tile([128, C], F32, tag="bpbc")
        wp_sb = pp.tile([128, KC, C], BF16, tag="wp")

        xt_r = dr["xt"][:].rearrange("p (kc t) -> p kc t", kc=KC)

        def _load_wa(tag, g, cols):
            wt = pw.tile([128, KC, cols], BF16, tag=tag, bufs=3)
            start = KC * _WAOFF[g]
            nc.sync.dma_start(
                out=wt[:],
                in_=dr["wa"][:, start : start + KC * cols].rearrange(
                    "p (kc n) -> p kc n", kc=KC
                ),
            )
            return wt

        # --- init DMAs in first-use order, split across both hwdge queues
        # (SP and ACT) so the first qk chain starts ~2.5us earlier ---
        wtp = {0: _load_wa("wtp", 2, 256)}
        nc.sync.dma_start(out=xt_sb[:, 0:3, 0:512], in_=xt_r[:, 0:3, 0:512])
        nc.scalar.dma_start(out=xt_sb[:, 3:6, 0:512], in_=xt_r[:, 3:6, 0:512])
        nc.scalar.dma_start(out=xt_sb[:, :, 512:1024], in_=xt_r[:, :, 512:1024])
        nc.sync.dma_start(out=cf32[:], in_=dr["cf32"][:])
        nc.sync.dma_start(out=cbf[:], in_=dr["cbf"][:])
        wtv = [_load_wa("wtv", 0, 384), _load_wa("wtv", 1, 384)]
        wtp[1] = _load_wa("wtp", 3, 256)
        nc.scalar.dma_start(out=bvbc_sb[:], in_=dr["bvbc"][:])
        nc.scalar.dma_start(out=bpbc_sb[:], in_=dr["bpbc"][:])
        # ones column of v via host DMA (DVE memset miscompiled on HW)
        nc.scalar.dma_start(
            out=v_sb[:, :, :, 64:65],
            in_=dr["vones"][:].rearrange("p (a b o) -> p a b o", a=NT, b=H),
        )

        def emit_qk(i):
            wt = wtp[i]
            # pj-serial chains: ldweights are free in the cost model, and
            # one-live-chain keeps the 2-buf psum pool rotation slack.
            # At i==0 run both pj0 chains first: the second xt half is still
            # in flight on the ACT dma queue.
            if i == 0:
                order = [(w_, pj) for pj in range(2) for w_ in range(2)]
            else:
                order = [(w_, pj) for w_ in range(2) for pj in range(2)]
            for which, pj in order:
                m = i + 6 * which
                ps = pb.tile([128, 512], F32, tag="pb", name=f"ps_{which}_{pj}")
                for kc in range(KC):
                    nc.tensor.matmul(
                        ps[:],
                        wt[:, kc, which * 128 : which * 128 + 128],
                        xt_sb[:, kc, pj * 512 : (pj + 1) * 512],
                        start=(kc == 0),
                        stop=(kc == KC - 1),
                    )
                w = slice(pj * 512, (pj + 1) * 512)
                # psum drain + fused bias (GPSIMD cannot access PSUM)
                nc.vector.tensor_scalar_add(
                    qkT[:, m, w], ps[:], bqk_sb[:, m : m + 1]
                )

        # rope rotation = adjacent-partition swap; the sign lives in the
        # host-packed signed sin table.
        swap_mask = [i ^ 1 for i in range(32)]

        def emit_rope(i):
            # shuffle on DVE (only engine with stream_shuffle); the three
            # elementwise ops run on the otherwise-idle Pool engine (all-SBUF
            # operands, so GPSIMD is legal here)
            for m in (i, 6 + i):
                shf = pat.tile([128, T], BF16, tag="shf", bufs=2)
                nc.vector.stream_shuffle(shf[:], qkT[:, m, :], swap_mask)
                t1 = pat.tile([128, T], BF16, tag="t1", bufs=2)
                nc.gpsimd.tensor_mul(t1[:], qkT[:, m, :], cos_sb[:])
                t2 = pat.tile([128, T], BF16, tag="t2", bufs=2)
                nc.gpsimd.tensor_mul(t2[:], shf[:], sin_sb[:])
                nc.gpsimd.tensor_add(qkT[:, m, :], t1[:], t2[:])

        es_store = {}

        def emit_s(i, tkcs):
            qv, kv = i, 6 + i
            for tkc in tkcs:
                lo = 128 * tkc
                width = T - lo
                for hh in range(2):
                    b0 = 64 * hh
                    st = psc.tile([128, 1024], F32, tag="psc")
                    for off in range(0, width, 512):
                        w = min(512, width - off)
                        nc.tensor.matmul(
                            st[:, off : off + w],
                            qkT[b0 : b0 + 64, kv, lo : lo + 128],
                            qkT[b0 : b0 + 64, qv, lo + off : lo + off + w],
                            start=True,
                            stop=True,
                        )
                    if width > 512:
                        es = pes.tile([128, 1024], BF16, tag="esw")
                    else:
                        es = pesn.tile([128, 512], BF16, tag="esn")
                    nc.scalar.activation(
                        es[:, 0:width], st[:, 0:width], AFT.Exp, scale=0.125
                    )
                    # causal mask: zero the upper triangle (tk > tq) of the
                    # diagonal block, in place, on Pool (all-SBUF: legal)
                    nc.gpsimd.tensor_mul(es[:, 0:128], es[:, 0:128], tri_sb)
                    es_store[(hh, tkc)] = es

        def emit_v(vg, ts):
            wt = wtv[vg]
            for t in ts:
                ps = pb.tile([128, 512], F32, tag="pb")
                for kc in range(KC):
                    nc.tensor.matmul(
                        ps[:, 0:384],
                        xt_sb[:, kc, t * 128 : (t + 1) * 128],
                        wt[:, kc, :],
                        start=(kc == 0),
                        stop=(kc == KC - 1),
                    )
                nc.vector.tensor_add(
                    v_sb[:, t, 6 * vg : 6 * vg + 6, 0:64],
                    ps[:, 0:384].rearrange("p (h d) -> p h d", h=6),
                    bvbc_sb[:, vg * 384 : (vg + 1) * 384].rearrange(
                        "p (h d) -> p h d", h=6
                    ),
                )

        def emit_proj_m(m):
            for piece in range(2):
                pw_ = slice(piece * 384, (piece + 1) * 384)
                po = pb.tile([128, 512], F32, tag="pb")
                for kc in range(KC):
                    nc.tensor.matmul(
                        po[:, 0:384],
                        yT[:, kc, m * 128 : (m + 1) * 128],
                        wp_sb[:, kc, pw_],
                        start=(kc == 0),
                        stop=(kc == KC - 1),
                    )
                osb = pbs.tile([128, 384], F32, tag="ob", bufs=3)
                nc.vector.tensor_add(osb[:], po[:, 0:384], bpbc_sb[:, pw_])
                nc.sync.dma_start(
                    out=dr["out"][m * 128 : (m + 1) * 128, pw_], in_=osb[:]
                )

        def emit_y(i, qbs, final):
            for qb in qbs:
                for hh in range(2):
                    h = 2 * i + hh
                    yp = py.tile([128, 512], F32, tag="py")
                    for j, tkc in enumerate(range(qb + 1)):
                        rel = qb * 128 - 128 * tkc
                        es = es_store[(hh, tkc)]
                        nc.tensor.matmul(
                            yp[:, 0:65],
                            es[:, rel : rel + 128],
                            v_sb[:, tkc, h, :],
                            start=(j == 0),
                            stop=(j == qb),
                        )
                    rinv = pbs.tile([128, 1], F32, tag="rinv", bufs=6)
                    nc.vector.reciprocal_approx_fast(
                        out=rinv[:], in_=yp[:, 64:65]
                    )
                    nc.vector.tensor_scalar_mul(
                        ynat[:, qb, h * 64 : (h + 1) * 64], yp[:, 0:64], rinv[:]
                    )
                # channels of pair i are exactly contraction chunk kc=i:
                # transpose this pair's [tq, 128ch] block on the DMA xbar
                nc.sync.dma_start_transpose(
                    out=yT[:, i, qb * 128 : (qb + 1) * 128],
                    in_=ynat[:, qb, i * 128 : (i + 1) * 128],
                )
                if final and 3 in phases and qb >= 2:
                    emit_proj_m(qb - 2)

        # ---------------- pipelined qkv + attention ----------------
        if 1 not in phases:
            return
        emit_qk(0)
        emit_rope(0)
        for i in range(1, 7):
            if i + 1 < 6:
                wtp[i + 1] = _load_wa("wtp", 2 + (i + 1), 256)
            if i == 4:
                nc.sync.dma_start(
                    out=wp_sb[:],
                    in_=dr["wp"][:].rearrange("p (kc n) -> p kc n", kc=KC),
                )
            if 2 in phases:
                emit_s(i - 1, range(0, 2))
            # v is spread over iterations 1-3 as PE filler between the
            # ACT-bound score bursts (vg0 fully before y(0); vg1 before y(3))
            if i == 1:
                emit_v(0, range(NT))
                emit_v(1, range(0, 2))
            elif i == 2:
                emit_v(1, range(2, 5))
            elif i == 3:
                emit_v(1, range(5, NT))
            if i < 6:
                emit_qk(i)
                emit_rope(i)
            if 2 in phases:
                final = i == 6
                # qb-level interleave: y(qb-2) fills PE while ACT drains the
                # exp of s(qb)
                for qb in range(2, NT):
                    emit_s(i - 1, [qb])
                    emit_y(i - 1, [qb - 2], final)
                emit_y(i - 1, [NT - 2], final)
                emit_y(i - 1, [NT - 1], final)
                if final and 3 in phases:
                    emit_proj_m(NT - 2)
                    emit_proj_m(NT - 1)

        if _DEBUG:
            nc.sync.dma_start(
                out=dr["dqk"][:], in_=qkT[:].rearrange("p a b -> p (a b)")
            )
            nc.sync.dma_start(
                out=dr["dv"][:], in_=v_sb[:].rearrange("p a b c -> p (a b c)")
            )
            nc.sync.dma_start(
                out=dr["dyt"][:], in_=yT[:].rearrange("p a b -> p (a b)")
            )


def _build_program(loop_n=None, phases=(1, 2, 3)):
    import concourse.bacc as bacc
    import concourse.tile as tile
    from concourse import mybir

    F32 = mybir.dt.float32
    BF16 = mybir.dt.bfloat16

    nc = bacc.Bacc(None, target_bir_lowering=False, debug=False)

    dr = {
        "xt": nc.dram_tensor("xt", [128, KC * T], BF16, kind="ExternalInput"),
        "wa": nc.dram_tensor("wa", [128, KC * _WATOT], BF16, kind="ExternalInput"),
        "bvbc": nc.dram_tensor("bvbc", [128, C], BF16, kind="ExternalInput"),
        "wp": nc.dram_tensor("wp", [128, KC * C], BF16, kind="ExternalInput"),
        "bpbc": nc.dram_tensor("bpbc", [128, C], F32, kind="ExternalInput"),
        "cbf": nc.dram_tensor("cbf", [128, 2 * T + 128], BF16, kind="ExternalInput"),
        "cf32": nc.dram_tensor("cf32", [128, 12], F32, kind="ExternalInput"),
        "vones": nc.dram_tensor("vones", [128, NT * H], BF16, kind="ExternalInput"),
        "out": nc.dram_tensor("out", [T, C], F32, kind="ExternalOutput"),
    }
    if _DEBUG:
        dr["dqk"] = nc.dram_tensor("dqk", [128, 12 * T], BF16, kind="ExternalOutput")
        dr["dv"] = nc.dram_tensor("dv", [128, NT * H * 65], BF16, kind="ExternalOutput")
        dr["dyt"] = nc.dram_tensor("dyt", [128, KC * T], BF16, kind="ExternalOutput")

    with tile.TileContext(nc) as tc:
        if loop_n is None:
            _emit_body(nc, tc, dr, phases)
        else:
            with tc.For_i(0, loop_n, 1):
                _emit_body(nc, tc, dr, phases)

    nc.compile()
    return nc


def _host_constants():
    """Constant tables shipped to every core."""
    inv_freq = (1.0 / (10000.0 ** (np.arange(0, D, 2, dtype=np.float32) / D))).astype(
        np.float32
    )
    tpos = np.arange(T, dtype=np.float32)
    freqs = tpos[None, :] * inv_freq[:, None]  # [32, T]
    cos32 = np.cos(freqs).astype(np.float32)
    sin32 = np.sin(freqs).astype(np.float32)
    cosT = np.repeat(cos32, 2, axis=0)  # [64, T], channel d -> freq d//2
    sinT = np.repeat(sin32, 2, axis=0)
    cosT = np.concatenate([cosT, cosT], axis=0)  # [128, T]: two head copies
    sinT = np.concatenate([sinT, sinT], axis=0)

    # rotation = adjacent-row swap; fold the signs into the sin table:
    # rot[2i] = -q[2i+1]*sin, rot[2i+1] = +q[2i]*sin
    sinS = sinT.copy()
    sinS[0::2, :] *= -1.0

    import ml_dtypes

    # multiplicative keep-mask in s^T orientation: keep tk <= tq
    tri = np.triu(np.ones((128, 128), dtype=np.float32)).astype(ml_dtypes.bfloat16)
    return cosT, sinS, tri


def _input_maps(x, W_attn, b_attn, W_proj, b_proj):
    import ml_dtypes

    BF = ml_dtypes.bfloat16
    cosT, sinS, tri_b = _host_constants()

    # wa: [C, 3C] -> [128, kc, n], columns packed as [vA | vB | pair0..pair5]
    wa = W_attn.reshape(KC, 128, 3 * C).transpose(1, 0, 2)  # [128, kc, 3C]
    groups = [wa[:, :, 2 * C : 2 * C + 384], wa[:, :, 2 * C + 384 : 3 * C]]
    for hp in range(6):
        groups.append(wa[:, :, hp * 128 : (hp + 1) * 128])  # q pair
        groups.append(wa[:, :, C + hp * 128 : C + (hp + 1) * 128])  # k pair
    # merge each pair's q+k into one 256-col group
    packed = [groups[0], groups[1]] + [
        np.concatenate([groups[2 + 2 * hp], groups[3 + 2 * hp]], axis=2)
        for hp in range(6)
    ]
    wa_g = np.concatenate([g.reshape(128, -1) for g in packed], axis=1)
    assert wa_g.shape[1] == KC * _WATOT

    wp = W_proj.reshape(KC, 128, C).transpose(1, 0, 2).reshape(128, KC * C)

    bqk = b_attn[: 2 * C].reshape(12, 128).T.astype(np.float32)
    cbf = np.concatenate(
        [cosT.astype(BF), sinS.astype(BF), tri_b],
        axis=1,
    )
    cf32 = np.ascontiguousarray(bqk)
    shared = {
        "wa": np.ascontiguousarray(wa_g.astype(BF)),
        "bvbc": np.ascontiguousarray(
            np.broadcast_to(b_attn[2 * C :].astype(BF), (128, C))
        ),
        "wp": np.ascontiguousarray(wp.astype(BF)),
        "bpbc": np.ascontiguousarray(np.broadcast_to(b_proj, (128, C))),
        "cbf": np.ascontiguousarray(cbf),
        "cf32": cf32,
        "vones": np.ones((128, NT * H), dtype=BF),
    }
    out = []
    for b in range(B):
        xt = (
            x[b].T.reshape(KC, 128, T).transpose(1, 0, 2).reshape(128, KC * T)
        )  # [128, KC*T]
        out.append(dict(shared, xt=np.ascontiguousarray(xt.astype(BF))))
    return out


def kernel(x, W_attn, b_attn, W_proj, b_proj):
    global _prog
    from concourse.bass_utils import run_bass_kernel_spmd

    if _prog is None:
        _prog = _build_program()

    x = np.asarray(x, dtype=np.float32)
    W_attn = np.asarray(W_attn, dtype=np.float32)
    b_attn = np.asarray(b_attn, dtype=np.float32)
    W_proj = np.asarray(W_proj, dtype=np.float32)
    b_proj = np.asarray(b_proj, dtype=np.float32)

    in_maps = _input_maps(x, W_attn, b_attn, W_proj, b_proj)
    # first post-load execution shows cold-start wobble in some ucode
    # engines; run once to warm up, return the steady-state result
    # retry guard: transient device errors (NRT unrecoverable) have been
    # observed on this fleet; a failed attempt costs only wall-clock
    res = None
    for attempt in range(3):
        try:
            run_bass_kernel_spmd(_prog, in_maps, list(range(N_CORES)))
            res = run_bass_kernel_spmd(_prog, in_maps, list(range(N_CORES)))
            break
        except Exception:
            if attempt == 2:
                raise
    out = np.stack([res.results[b]["out"] for b in range(B)], axis=0)
    return out.astype(np.float32)
